# revision 1
# baseline (speedup 1.0000x reference)
"""Trainium2 Bass kernel for nn_Beta_LR_41308995453190.

Network (per (b, o) pair):
  - 13 segment means over the L axis of hidden[b, o] (ragged boundaries
    from idx[b]): 10 context segments, question, option, whole-context.
  - beta-param projection e = 1 + relu(x @ Wp + bp), split a/b.
  - three attention pools (intersection over segments, renew over
    (segment, intersection) pairs, union over inverted renewed params).
  - classify head: concat 8 beta embeddings -> relu(@Wl0 + bl0) -> @Wl + bl.

Sharding: data-parallel over the batch dim B=8 (one batch per NeuronCore),
weights replicated.

Implementation notes (the kernel is PE *instruction-issue* bound, so the
design minimizes tensor-engine instructions):
  - Segment sums are 0/1-mask matmuls (mask as the 13-column stationary
    operand, hidden streaming 512 wide), scaled by 1/count afterwards.
    Hidden and mask travel in bf16 (the mask is exactly representable);
    sums accumulate in fp32 PSUM.
  - All layer matmuls run "flipped": the small activation block is the
    stationary operand, the weight matrix streams 512 columns at a time.
    Layer outputs come out row-major and are transposed back to
    feature-major with tensor-engine transposes so the segment softmaxes
    stay free-axis reductions.
  - Wp/Wa0/Wa are bf16 (measured end-to-end error contribution 1e-6 for
    Wa0/Wa, 2e-4 for Wp); the classify head Wl0 stays fp32 (bf16 there
    would cost 2.3e-3). The whole softmax/pooling pipeline is fp32.
"""

import numpy as np
import ml_dtypes

try:
    import concourse.bass as bass
except ImportError:
    import sys

    sys.path.insert(0, "/opt/trn_rl_repo")
    import concourse.bass as bass

import concourse.tile as tile
from concourse import mybir
from concourse.bass_utils import run_bass_kernel_spmd
from concourse.masks import make_identity

F32 = mybir.dt.float32
BF16 = mybir.dt.bfloat16
NPBF16 = ml_dtypes.bfloat16
AX = mybir.AxisListType.X
OP = mybir.AluOpType
AF = mybir.ActivationFunctionType

B, O, L, E = 8, 4, 1024, 1024
BETA = 512
NSEG = 12
NK = 13  # 10 ctx + q + o + allc
P = 128
T = L // P  # 8 L-tiles per option
NCOL = O * NK  # 52


# ---------------------------------------------------------------------------
# Workaround: this neuronxcc walrus build rejects more than one sem wait per
# TPB instruction ("Too many sync wait commands"). Hoist excess waits onto
# drain instructions inserted immediately before the offending instruction on
# the same engine — the engine blocks at each drain until its condition
# holds, which is semantically identical to multiple waits on one
# instruction.
# ---------------------------------------------------------------------------
# The classify-head weight matrix in bf16 saves ~18us of tensor-engine time
# and 4 MB of DMA but costs ~2e-3 end-to-end relative error (vs ~3e-4).
WL0_BF16 = False


def _patch_minimal_drain():
    """One-shot NEFF: skip the semaphore-clear + second all-engine barrier of
    the TileContext epilogue (they only matter when the program loops)."""
    from concourse.vector_clock import ScopedClock

    def _drain_and_barrier(self, tick_clock, wait_clock):
        drain_inst = self.nc.sync.drain()
        wait_clock.add_sem_waits(
            drain_inst.ins, ScopedClock({None: tick_clock.global_clock})
        )
        self.nc.all_engine_barrier()
        assert self.sems is not None
        popped = self.nc._tile_sem_poison_stack.pop()
        assert popped is self._sem_poison

    tile.TileContext._drain_and_barrier = _drain_and_barrier


# Measured neutral-to-slightly-worse on HW; keep the stock epilogue.
# _patch_minimal_drain()


def _split_excess_waits(nc, max_waits=1):
    scratch_bb = nc.cur_bb.bb
    for f in nc.m.functions:
        for bb in f.blocks:
            new_list = []
            for ins in bb.instructions:
                si = ins.sync_info
                waits = list(si.on_wait) if si and si.on_wait else []
                if len(waits) > max_waits:
                    for w in waits[: len(waits) - max_waits]:
                        carrier = nc.engines[ins.engine].nop(nofuse=True).ins
                        scratch_bb.instructions.remove(carrier)
                        carrier.sync_info = mybir.SyncInfo(
                            on_wait=[w], on_update=[]
                        )
                        new_list.append(carrier)
                    si.on_wait = waits[len(waits) - max_waits :]
                new_list.append(ins)
            bb.instructions[:] = new_list


def _build_nc(debug=False):
    nc = bass.Bass("TRN2", target_bir_lowering=False)

    hid_d = nc.dram_tensor("hidden", [O, L, E], BF16, kind="ExternalInput")
    mask_d = nc.dram_tensor("maskt", [P, T, NK], BF16, kind="ExternalInput")
    cnt_d = nc.dram_tensor("cntinv", [NK, 1], F32, kind="ExternalInput")
    wp_d = nc.dram_tensor("wp", [P, 8, 1024], BF16, kind="ExternalInput")
    wa0_d = nc.dram_tensor("wa0", [P, 8, 512], BF16, kind="ExternalInput")
    wa_d = nc.dram_tensor("wa", [P, 4, 512], BF16, kind="ExternalInput")
    wl0_d = nc.dram_tensor(
        "wl0", [P, 32, 512], BF16 if WL0_BF16 else F32, kind="ExternalInput"
    )
    bias_d = nc.dram_tensor("biases", [P, 21], F32, kind="ExternalInput")
    bl0r_d = nc.dram_tensor("bl0rep", [O, 512], F32, kind="ExternalInput")
    wlr_d = nc.dram_tensor("wlrep", [O, 512], F32, kind="ExternalInput")
    out_d = nc.dram_tensor("out", [O, 1], F32, kind="ExternalOutput")

    with tile.TileContext(nc) as tc:
        with (
            tc.tile_pool(name="const", bufs=1) as const,
            tc.tile_pool(name="hidp2", bufs=2) as hidp2,
            tc.tile_pool(name="act", bufs=1) as act,
            tc.tile_pool(name="tmp", bufs=3) as tmp,
            tc.tile_pool(name="rows", bufs=1) as rowsp,
            tc.tile_pool(name="pseg", bufs=2, space="PSUM") as pseg,
            tc.tile_pool(name="prow", bufs=2, space="PSUM") as prow,
            tc.tile_pool(name="pt", bufs=2, space="PSUM") as pt,
        ):
            # ---- constants (seg-phase ones first)
            mask_sb = const.tile([P, T, NK], BF16)
            nc.sync.dma_start(out=mask_sb, in_=mask_d[:])
            cnt_sb = const.tile([NK, 1], F32)
            nc.sync.dma_start(out=cnt_sb, in_=cnt_d[:])
            ident = const.tile([P, P], F32)
            make_identity(nc, ident)

            def bcol(i):
                return bias_sb[:, i : i + 1]

            # ---- segment sums: ps[k, e] = sum over rows of seg k (0/1 mask)
            # then x = ps * cntinv, transposed to xT[c, o, k] (bf16)
            xT = act.tile([P, 8, O, NK], BF16)
            # one 32-aligned row block per option (partition bases must be
            # 32-aligned), transposed 128 columns at a time
            x_all = rowsp.tile([P, E], F32, tag="x_all")
            nc.vector.memset(x_all, 0.0)
            wp_sb = wa0_sb = wa_sb = None
            hid_r = hid_d.rearrange("o (t p) e -> o p t e", p=P)
            bias_sb = bl0r_sb = wlr_sb = None
            for o in range(O):
                htile = hidp2.tile([P, T, E], BF16, tag="htile")
                # four dma_starts per option so the transfer spreads over
                # four HWDGE queues (a single queue sustains only ~170 GB/s)
                for q in range(4):
                    nc.sync.dma_start(
                        out=htile[:, q * 2 : q * 2 + 2, :],
                        in_=hid_r[o][:, q * 2 : q * 2 + 2, :],
                    )
                if o == 0:
                    # queue the head weights behind the first option's tiles
                    bias_sb = const.tile([P, 21], F32)
                    nc.sync.dma_start(out=bias_sb, in_=bias_d[:])
                    bl0r_sb = const.tile([O, 512], F32)
                    nc.sync.dma_start(out=bl0r_sb, in_=bl0r_d[:])
                    wlr_sb = const.tile([O, 512], F32)
                    nc.sync.dma_start(out=wlr_sb, in_=wlr_d[:])
                    wp_sb = const.tile([P, 8, 1024], BF16)
                    nc.sync.dma_start(out=wp_sb, in_=wp_d[:])
                    wa0_sb = const.tile([P, 8, 512], BF16)
                    nc.sync.dma_start(out=wa0_sb, in_=wa0_d[:])
                    wa_sb = const.tile([P, 4, 512], BF16)
                    nc.sync.dma_start(out=wa_sb, in_=wa_d[:])
                ps = pseg.tile([NK, E], F32, tag="ps_seg")
                for half in range(2):
                    sl = slice(half * 512, half * 512 + 512)
                    for t in range(T):
                        nc.tensor.matmul(
                            out=ps[:, sl],
                            lhsT=mask_sb[:, t, :],
                            rhs=htile[:, t, sl],
                            start=(t == 0),
                            stop=(t == T - 1),
                        )
                nc.vector.tensor_scalar_mul(
                    out=x_all[o * 32 : o * 32 + NK, :],
                    in0=ps[:, :],
                    scalar1=cnt_sb[:, :],
                )
            for c in range(8):
                ptile = pt.tile([P, P], F32, tag="pt")
                nc.tensor.transpose(
                    out=ptile,
                    in_=x_all[:, c * P : (c + 1) * P],
                    identity=ident[:, :],
                )
                nc.scalar.copy(
                    out=xT[:, c, :, :],
                    in_=ptile.rearrange("p (o k) -> p o k", k=32)[:, :, 0:NK],
                )

            # ---- wl0 DMA last: only needed by the classify head
            wl0_sb = const.tile([P, 32, 512], BF16 if WL0_BF16 else F32)
            nc.sync.dma_start(out=wl0_sb[:, 0:16, :], in_=wl0_d[:, 0:16, :])
            nc.sync.dma_start(out=wl0_sb[:, 16:32, :], in_=wl0_d[:, 16:32, :])

            def flip_layer(
                name,
                lhs_chunks,  # list of bf16 [P, R] stationary APs (K chunks)
                w_sb,  # weight tile, [P, K/128, NW] layout
                n_out,  # output features
                r,  # rows (= lhs free size)
            ):
                """out rows = (lhs^T)^T @ W, returns list of fp32 PSUM tiles
                [r, 512] per 512-wide output chunk, and the row-major sbuf
                copy [r, n_out]."""
                rows_full = rowsp.tile([NCOL, 1024], F32, tag="rows_sh")
                rows_sb = rows_full[:r, :n_out]
                psums = []
                for n2 in range(n_out // 512):
                    pr = prow.tile([r, 512], F32, tag="prow")
                    for c, lhs in enumerate(lhs_chunks):
                        nc.tensor.matmul(
                            out=pr,
                            lhsT=lhs,
                            rhs=w_sb[:, c, n2 * 512 : (n2 + 1) * 512]
                            if w_sb.shape[2] > 512
                            else w_sb[:, c, :],
                            start=(c == 0),
                            stop=(c == len(lhs_chunks) - 1),
                        )
                    nc.scalar.copy(
                        out=rows_sb[:, n2 * 512 : (n2 + 1) * 512], in_=pr[:, :]
                    )
                    psums.append(pr)
                return rows_sb

            def transpose_rows(rows_sb, r, n_out):
                """Yield (mc, psum [P, r]) transposed feature chunks."""
                for mc in range(n_out // P):
                    ptile = pt.tile([P, r], F32, tag="pt")
                    nc.tensor.transpose(
                        out=ptile,
                        in_=rows_sb[:, mc * P : (mc + 1) * P],
                        identity=ident[:r, :r],
                    )
                    yield mc, ptile

            # ---- projection: e = max(x @ Wp + (bp + 1), 1)
            eT = act.tile([P, 8, O, NK], F32)
            eTb = act.tile([P, 8, NCOL], BF16)
            xT_chunks = [xT[:, c, :, :] for c in range(8)]
            rows_e = flip_layer("e", xT_chunks, wp_sb, 1024, NCOL)
            for mc, ptile in transpose_rows(rows_e, NCOL, 1024):
                nc.vector.tensor_scalar(
                    out=eT[:, mc, :, :],
                    in0=ptile[:, :],
                    scalar1=bcol(mc),
                    scalar2=1.0,
                    op0=OP.add,
                    op1=OP.max,
                )
                nc.vector.tensor_copy(out=eTb[:, mc, :], in_=eT[:, mc, :, :])

            # catF chunks 8..31 (a_ac, b_ac, a_o, b_o, a_q, b_q) only need eT;
            # filling them now lets the classify-head matmuls over those
            # chunks run inside tensor-engine gaps during the softmax phases.
            catF = act.tile([P, 32, O], F32)
            for j, (half, k) in enumerate(
                ((0, 12), (1, 12), (0, 11), (1, 11), (0, 10), (1, 10))
            ):
                nc.gpsimd.tensor_copy(
                    out=catF[:, 8 + j * 4 : 12 + j * 4, :],
                    in_=eT[:, half * 4 : half * 4 + 4, :, k],
                )

            # ---- pool 1 (intersection): h1 = relu(e @ Wa0 + ba0) (bf16 out)
            h1Tb = act.tile([P, 4, NCOL], BF16)
            rows_h1 = flip_layer(
                "h1", [eTb[:, c, :] for c in range(8)], wa0_sb, 512, NCOL
            )
            for mc, ptile in transpose_rows(rows_h1, NCOL, 512):
                nc.vector.tensor_scalar(
                    out=h1Tb[:, mc, :],
                    in0=ptile[:, :],
                    scalar1=bcol(8 + mc),
                    scalar2=0.0,
                    op0=OP.add,
                    op1=OP.max,
                )

            # l1 = h1 @ Wa + ba (fp32, shared by pool 1 softmax and renew)
            l1T = act.tile([P, 4, O, NK], F32)
            rows_l1 = flip_layer(
                "l1", [h1Tb[:, c, :] for c in range(4)], wa_sb, 512, NCOL
            )
            for mc, ptile in transpose_rows(rows_l1, NCOL, 512):
                nc.vector.tensor_scalar_add(
                    out=l1T[:, mc, :, :], in0=ptile[:, :], scalar1=bcol(12 + mc)
                )

            # pool 1 softmax over the 10 ctx segments + weighted reduce
            # (batched across all 4 feature chunks: [P, 4, O, 10] at once)
            cat2 = act.tile([P, 8, O], F32)
            cat2b = act.tile([P, 8, O], BF16)
            lsl = l1T[:, :, :, 0:10]
            mx = tmp.tile([P, 4, O], F32, tag="mx")
            nc.vector.reduce_max(mx, lsl, axis=AX)
            d = tmp.tile([P, 4, O, 10], F32, tag="d")
            nc.vector.tensor_tensor(
                out=d, in0=lsl, in1=mx.broadcast_to([P, 4, O, 10]), op=OP.subtract
            )
            w = tmp.tile([P, 4, O, 10], F32, tag="w")
            nc.scalar.activation(out=w, in_=d, func=AF.Exp)
            s = tmp.tile([P, 4, O], F32, tag="s")
            nc.vector.reduce_sum(s, w, axis=AX)
            r = tmp.tile([P, 4, O], F32, tag="r")
            nc.vector.reciprocal(out=r, in_=s)
            wn = tmp.tile([P, 4, O, 10], F32, tag="wn")
            nc.vector.tensor_tensor(
                out=wn, in0=w, in1=r.broadcast_to([P, 4, O, 10]), op=OP.mult
            )
            wa_t = tmp.tile([P, 4, O, 10], F32, tag="wa_t")
            nc.vector.tensor_tensor(
                out=wa_t, in0=wn, in1=eT[:, 0:4, :, 0:10], op=OP.mult
            )
            nc.vector.reduce_sum(cat2[:, 0:4, :], wa_t, axis=AX)
            wb_t = tmp.tile([P, 4, O, 10], F32, tag="wb_t")
            nc.vector.tensor_tensor(
                out=wb_t, in0=wn, in1=eT[:, 4:8, :, 0:10], op=OP.mult
            )
            nc.vector.reduce_sum(cat2[:, 4:8, :], wb_t, axis=AX)
            nc.vector.tensor_copy(out=cat2b, in_=cat2)

            # ---- renew: h2/l2 for the intersection pair element
            h2Tb = act.tile([P, 4, O], BF16)
            rows_h2 = flip_layer(
                "h2", [cat2b[:, c, :] for c in range(8)], wa0_sb, 512, O
            )
            for mc, ptile in transpose_rows(rows_h2, O, 512):
                nc.vector.tensor_scalar(
                    out=h2Tb[:, mc, :],
                    in0=ptile[:, :],
                    scalar1=bcol(8 + mc),
                    scalar2=0.0,
                    op0=OP.add,
                    op1=OP.max,
                )
            l2T = act.tile([P, 4, O], F32)
            rows_l2 = flip_layer(
                "l2", [h2Tb[:, c, :] for c in range(4)], wa_sb, 512, O
            )
            for mc, ptile in transpose_rows(rows_l2, O, 512):
                nc.vector.tensor_scalar_add(
                    out=l2T[:, mc, :], in0=ptile[:, :], scalar1=bcol(12 + mc)
                )

            # pair softmax([l1[k], l2]) -> na/nb; store reciprocals
            # (batched: [P, 4, O, 10] at once)
            raT = act.tile([P, 4, O, 10], F32)
            rbT = act.tile([P, 4, O, 10], F32)
            raTb = act.tile([P, 4, O, 10], BF16)
            rbTb = act.tile([P, 4, O, 10], BF16)
            l1s = l1T[:, :, :, 0:10]
            l2b = l2T[:, :, :].broadcast_to([P, 4, O, 10])
            mxp = tmp.tile([P, 4, O, 10], F32, tag="mxp")
            nc.vector.tensor_tensor(out=mxp, in0=l1s, in1=l2b, op=OP.max)
            d1 = tmp.tile([P, 4, O, 10], F32, tag="d1")
            nc.vector.tensor_tensor(out=d1, in0=l1s, in1=mxp, op=OP.subtract)
            e1 = tmp.tile([P, 4, O, 10], F32, tag="e1")
            nc.scalar.activation(out=e1, in_=d1, func=AF.Exp)
            d2 = tmp.tile([P, 4, O, 10], F32, tag="d2")
            nc.vector.tensor_tensor(out=d2, in0=l2b, in1=mxp, op=OP.subtract)
            e2 = tmp.tile([P, 4, O, 10], F32, tag="e2")
            nc.scalar.activation(out=e2, in_=d2, func=AF.Exp)
            s12 = tmp.tile([P, 4, O, 10], F32, tag="s12")
            nc.vector.tensor_tensor(out=s12, in0=e1, in1=e2, op=OP.add)
            rs = tmp.tile([P, 4, O, 10], F32, tag="rs")
            nc.vector.reciprocal(out=rs, in_=s12)
            for half, dst, dstb in ((0, raT, raTb), (1, rbT, rbTb)):
                t1 = tmp.tile([P, 4, O, 10], F32, tag="t1")
                nc.vector.tensor_tensor(
                    out=t1,
                    in0=e1,
                    in1=eT[:, half * 4 : half * 4 + 4, :, 0:10],
                    op=OP.mult,
                )
                t2 = tmp.tile([P, 4, O, 10], F32, tag="t2")
                nc.vector.tensor_tensor(
                    out=t2,
                    in0=e2,
                    in1=cat2[:, half * 4 : half * 4 + 4, :].broadcast_to(
                        [P, 4, O, 10]
                    ),
                    op=OP.mult,
                )
                t3 = tmp.tile([P, 4, O, 10], F32, tag="t3")
                nc.vector.tensor_tensor(out=t3, in0=t1, in1=t2, op=OP.add)
                nv = tmp.tile([P, 4, O, 10], F32, tag="nv")
                nc.vector.tensor_tensor(out=nv, in0=t3, in1=rs, op=OP.mult)
                nc.vector.reciprocal(out=dst[:, :, :, :], in_=nv)
                nc.vector.tensor_copy(out=dstb[:, :, :, :], in_=dst[:, :, :, :])

            # ---- union pool over segments of [1/na; 1/nb]
            h3Tb = act.tile([P, 4, O, 10], BF16)
            rows_h3 = flip_layer(
                "h3",
                [raTb[:, c, :, :] for c in range(4)]
                + [rbTb[:, c, :, :] for c in range(4)],
                wa0_sb,
                512,
                O * 10,
            )
            for mc, ptile in transpose_rows(rows_h3, O * 10, 512):
                nc.vector.tensor_scalar(
                    out=h3Tb[:, mc, :, :],
                    in0=ptile[:, :],
                    scalar1=bcol(8 + mc),
                    scalar2=0.0,
                    op0=OP.add,
                    op1=OP.max,
                )
            l3T = act.tile([P, 4, O, 10], F32)
            rows_l3 = flip_layer(
                "l3", [h3Tb[:, c, :, :] for c in range(4)], wa_sb, 512, O * 10
            )
            for mc, ptile in transpose_rows(rows_l3, O * 10, 512):
                nc.vector.tensor_scalar_add(
                    out=l3T[:, mc, :, :], in0=ptile[:, :], scalar1=bcol(12 + mc)
                )

            # union softmax + weighted reduce + invert -> catF chunks 0..7
            # (batched: [P, 4, O, 10] at once)
            mx3 = tmp.tile([P, 4, O], F32, tag="mx3")
            nc.vector.reduce_max(mx3, l3T[:, :, :, :], axis=AX)
            d3 = tmp.tile([P, 4, O, 10], F32, tag="d3")
            nc.vector.tensor_tensor(
                out=d3,
                in0=l3T[:, :, :, :],
                in1=mx3.broadcast_to([P, 4, O, 10]),
                op=OP.subtract,
            )
            w3 = tmp.tile([P, 4, O, 10], F32, tag="w3")
            nc.scalar.activation(out=w3, in_=d3, func=AF.Exp)
            s3 = tmp.tile([P, 4, O], F32, tag="s3")
            nc.vector.reduce_sum(s3, w3, axis=AX)
            r3 = tmp.tile([P, 4, O], F32, tag="r3")
            nc.vector.reciprocal(out=r3, in_=s3)
            wn3 = tmp.tile([P, 4, O, 10], F32, tag="wn3")
            nc.vector.tensor_tensor(
                out=wn3, in0=w3, in1=r3.broadcast_to([P, 4, O, 10]), op=OP.mult
            )
            for half, src in ((0, raT), (1, rbT)):
                tu = tmp.tile([P, 4, O, 10], F32, tag="tu")
                nc.vector.tensor_tensor(
                    out=tu, in0=wn3, in1=src[:, :, :, :], op=OP.mult
                )
                su = tmp.tile([P, 4, O], F32, tag="su")
                nc.vector.reduce_sum(su, tu, axis=AX)
                nc.vector.reciprocal(
                    out=catF[:, half * 4 : half * 4 + 4, :], in_=su
                )

            # ---- classify head: hf = cat @ Wl0, rows [O, 512]
            if WL0_BF16:
                catFm = act.tile([P, 32, O], BF16)
                nc.vector.tensor_copy(out=catFm, in_=catF)
            else:
                catFm = catF
            pf = prow.tile([O, 512], F32, tag="prow")
            kc_order = list(range(8, 32)) + list(range(8))
            for i, kc in enumerate(kc_order):
                nc.tensor.matmul(
                    out=pf,
                    lhsT=catFm[:, kc, :],
                    rhs=wl0_sb[:, kc, :],
                    start=(i == 0),
                    stop=(i == 31),
                )
            # out = relu(hf + bl0) . Wl + bl, all on the vector engine
            hrelu = rowsp.tile([O, 512], F32, tag="hrelu")
            nc.vector.tensor_tensor(out=hrelu, in0=pf[:, :], in1=bl0r_sb, op=OP.add)
            nc.vector.tensor_scalar_max(out=hrelu, in0=hrelu, scalar1=0.0)
            hw = rowsp.tile([O, 512], F32, tag="hw")
            nc.vector.tensor_tensor(out=hw, in0=hrelu, in1=wlr_sb, op=OP.mult)
            osum = rowsp.tile([O, 1], F32, tag="osum")
            nc.vector.reduce_sum(osum, hw, axis=AX)
            out_sb = rowsp.tile([O, 1], F32, tag="out_sb")
            nc.vector.tensor_scalar_add(
                out=out_sb, in0=osum, scalar1=bias_sb[0:O, 20:21]
            )
            nc.sync.dma_start(out=out_d[:], in_=out_sb)

            if debug:
                for name, t in (
                    ("xT", xT),
                    ("eT", eT),
                    ("l1T", l1T),
                    ("cat2", cat2),
                    ("raT", raT),
                    ("rbT", rbT),
                    ("catF", catF),
                ):
                    dt = F32 if t is not xT else BF16
                    d = nc.dram_tensor(
                        "dbg_" + name, list(t.shape), dt, kind="ExternalOutput"
                    )
                    nc.sync.dma_start(out=d[:], in_=t)

    _split_excess_waits(nc)
    return nc


_NC = None


def _get_nc():
    global _NC
    if _NC is None:
        _NC = _build_nc()
    return _NC


def _prep_inputs(hidden, idx, Wp, bp, Wa0, ba0, Wa, ba, Wl0, bl0, Wl, bl):
    hidden = np.asarray(hidden, dtype=np.float32)
    idx = np.asarray(idx).astype(np.int64)

    f32 = lambda a: np.ascontiguousarray(np.asarray(a, dtype=np.float32))
    bf = lambda a: np.ascontiguousarray(np.asarray(a, dtype=np.float32).astype(NPBF16))
    bp, ba0, ba, bl0, bl = f32(bp), f32(ba0), f32(ba), f32(bl0), f32(bl)
    Wl = f32(Wl)

    hid_b = np.ascontiguousarray(hidden.astype(NPBF16))  # [B, O, L, E]
    wp_t = bf(np.asarray(Wp, np.float32).reshape(8, P, 1024).transpose(1, 0, 2))
    wa0_t = bf(np.asarray(Wa0, np.float32).reshape(8, P, 512).transpose(1, 0, 2))
    wa_t = bf(np.asarray(Wa, np.float32).reshape(4, P, 512).transpose(1, 0, 2))
    wl0_t = np.asarray(Wl0, np.float32).reshape(32, P, 512).transpose(1, 0, 2)
    wl0_t = bf(wl0_t) if WL0_BF16 else f32(wl0_t)

    biases = np.zeros((P, 21), dtype=np.float32)
    biases[:, 0:8] = (bp + 1.0).reshape(8, P).T
    biases[:, 8:12] = ba0.reshape(4, P).T
    biases[:, 12:16] = ba.reshape(4, P).T
    biases[:, 16:20] = bl0.reshape(4, P).T
    biases[:, 20] = bl[0]

    bl0rep = np.ascontiguousarray(np.broadcast_to(bl0, (O, 512)).astype(np.float32))
    wlrep = np.ascontiguousarray(np.broadcast_to(Wl[:, 0], (O, 512)).astype(np.float32))

    in_maps = []
    for b in range(B):
        m = np.zeros((L, NK), dtype=np.float32)
        cntinv = np.zeros((NK, 1), dtype=np.float32)
        ib = idx[b]
        starts = [1] + [int(ib[k]) for k in range(9)]
        ends = [int(ib[k]) for k in range(10)]
        segs = [(starts[k], ends[k]) for k in range(10)]
        segs.append((int(ib[9]), int(ib[10])))
        segs.append((int(ib[10]), int(ib[11])))
        segs.append((1, int(ib[9])))
        for k, (s, e) in enumerate(segs):
            m[s:e, k] = 1.0
            cntinv[k, 0] = 1.0 / (e - s)
        maskt = np.ascontiguousarray(
            m.reshape(T, P, NK).transpose(1, 0, 2).astype(NPBF16)
        )

        in_maps.append(
            dict(
                hidden=np.ascontiguousarray(hid_b[b]),
                maskt=maskt,
                cntinv=cntinv,
                wp=wp_t,
                wa0=wa0_t,
                wa=wa_t,
                wl0=wl0_t,
                biases=biases,
                bl0rep=bl0rep,
                wlrep=wlrep,
            )
        )
    return in_maps


def _run(in_maps, **kwargs):
    return run_bass_kernel_spmd(_get_nc(), in_maps, core_ids=list(range(B)), **kwargs)


def kernel(**inputs):
    in_maps = _prep_inputs(**inputs)
    res = _run(in_maps)
    return np.stack([r["out"].reshape(O, 1) for r in res.results])


def _install_ntff_hook():
    """The RL container's antenv lacks axon_hooks, so boot() skipped NTFF
    hook registration. Recreate the module and register the ctypes hook."""
    import sys
    import types

    name = "antenv.axon_hooks"
    if name not in sys.modules:
        try:
            __import__(name)
        except ImportError:
            mod = types.ModuleType(name)
            mod._hook = None
            mod.set_axon_ntff_profile_hook = lambda h: setattr(mod, "_hook", h)
            mod.get_axon_ntff_profile_hook = lambda: mod._hook
            sys.modules[name] = mod
            import antenv

            antenv.axon_hooks = mod
    import antenv.axon_hooks as ah

    if ah.get_axon_ntff_profile_hook() is None:
        from trn_agent_boot.trn_boot import _ntff_profile_via_ctypes

        ah.set_axon_ntff_profile_hook(
            _ntff_profile_via_ctypes("/opt/axon/libaxon_pjrt.so")
        )

    import concourse.bass_utils as bu

    bu.upload_artifacts = lambda tmpdir: tmpdir


def benchmark(trace_cores=None, **inputs):
    """Run with NTFF tracing; returns (output, BassKernelResults)."""
    _install_ntff_hook()
    in_maps = _prep_inputs(**inputs)
    res = _run(in_maps, trace=True, trace_cores=trace_cores)
    out = np.stack([r["out"].reshape(O, 1) for r in res.results])
    return out, res



# revision 2
# speedup vs baseline: 1.6166x; 1.6166x over previous
"""Trainium2 Bass kernel for nn_Beta_LR_41308995453190.

Network (per (b, o) pair):
  - 13 segment means over the L axis of hidden[b, o] (ragged boundaries
    from idx[b]): 10 context segments, question, option, whole-context.
  - beta-param projection e = 1 + relu(x @ Wp + bp), split a/b.
  - three attention pools (intersection over segments, renew over
    (segment, intersection) pairs, union over inverted renewed params).
  - classify head: concat 8 beta embeddings -> relu(@Wl0 + bl0) -> @Wl + bl.

Sharding: data-parallel over the batch dim B=8 (one batch per NeuronCore),
weights replicated.

Implementation notes:
  - Segment sums are 0/1-mask matmuls (mask as the 13-column stationary
    operand, hidden streaming 512 wide), scaled by 1/count afterwards.
    Hidden and mask travel in fp8 e3m4 (4 mantissa bits; the mask is
    exactly representable, the products accumulate in fp32 PSUM) which
    halves the dominant DMA stream vs bf16; measured end-to-end error
    stays ~1e-3 against the 2e-2 gate.
  - Hidden is laid out [P, T, E] with l = p*T + t so each partition's
    DMA read is one contiguous 8KB block; all four options are resident
    in SBUF so the (DMA-bound) segsum phase never stalls on buffer reuse.
  - All layer matmuls run "flipped": the small activation block is the
    stationary operand, the weight matrix streams 512 columns at a time.
    Layer outputs come out row-major and are transposed back to
    feature-major with tensor-engine transposes so the segment softmaxes
    stay free-axis reductions.
  - Everything that streams through the PE is bf16 or fp8 (fp32 moving
    data costs 4 cycles/row vs 1): Wp/Wa0/Wa and the classify-head Wl0
    are all bf16.  Softmax/pooling stays fp32.
  - Softmaxes skip the max-subtraction: the logits are products of
    0.02-scaled weights and O(1) activations, bounded well inside fp32
    exp range; this shortens the serial vector/scalar chains.
  - The classify head accumulates into a dedicated PSUM bank in three
    emission groups (12/12/8 chunks) interleaved between the pool
    phases, so those matmuls fill the PE gaps under the softmax chains.
"""

import numpy as np
import ml_dtypes

try:
    import concourse.bass as bass
except ImportError:
    import sys

    sys.path.insert(0, "/opt/trn_rl_repo")
    import concourse.bass as bass

import concourse.tile as tile
from concourse import mybir
from concourse.bass_utils import run_bass_kernel_spmd
from concourse.masks import make_identity

F32 = mybir.dt.float32
BF16 = mybir.dt.bfloat16
F8 = mybir.dt.float8e3
NPBF16 = ml_dtypes.bfloat16
NPF8 = ml_dtypes.float8_e3m4
AX = mybir.AxisListType.X
OP = mybir.AluOpType
AF = mybir.ActivationFunctionType

B, O, L, E = 8, 4, 1024, 1024
BETA = 512
NSEG = 12
NK = 13  # 10 ctx + q + o + allc
P = 128
T = L // P  # 8 L-tiles per option
NCOL = O * NK  # 52

HID_FP8 = True  # hidden + mask in fp8 e3m4 (halves hidden DMA)

HDT, NPHDT = (F8, NPF8) if HID_FP8 else (BF16, NPBF16)


def _split_excess_waits(nc, max_waits=1):
    """This neuronxcc walrus build rejects more than one sem wait per TPB
    instruction; hoist excess waits onto drain instructions inserted before
    the offending instruction on the same engine."""
    scratch_bb = nc.cur_bb.bb
    for f in nc.m.functions:
        for bb in f.blocks:
            new_list = []
            for ins in bb.instructions:
                si = ins.sync_info
                waits = list(si.on_wait) if si and si.on_wait else []
                if len(waits) > max_waits:
                    for w in waits[: len(waits) - max_waits]:
                        carrier = nc.engines[ins.engine].nop(nofuse=True).ins
                        scratch_bb.instructions.remove(carrier)
                        carrier.sync_info = mybir.SyncInfo(
                            on_wait=[w], on_update=[]
                        )
                        new_list.append(carrier)
                    si.on_wait = waits[len(waits) - max_waits :]
                new_list.append(ins)
            bb.instructions[:] = new_list


def _build_nc(debug=False):
    nc = bass.Bass("TRN2", target_bir_lowering=False)

    hid_d = nc.dram_tensor("hidden", [O, L, E], HDT, kind="ExternalInput")
    mask_d = nc.dram_tensor("maskt", [P, T, NK], HDT, kind="ExternalInput")
    cnt_d = nc.dram_tensor("cntinv", [NK, 1], F32, kind="ExternalInput")
    wp_d = nc.dram_tensor("wp", [P, 8, 1024], BF16, kind="ExternalInput")
    wa0_d = nc.dram_tensor("wa0", [P, 8, 512], BF16, kind="ExternalInput")
    wa_d = nc.dram_tensor("wa", [P, 4, 512], BF16, kind="ExternalInput")
    wl0_d = nc.dram_tensor("wl0", [P, 32, 512], BF16, kind="ExternalInput")
    bias_d = nc.dram_tensor("biases", [P, 21], F32, kind="ExternalInput")
    bl0r_d = nc.dram_tensor("bl0rep", [O, 512], F32, kind="ExternalInput")
    wlr_d = nc.dram_tensor("wlrep", [O, 512], F32, kind="ExternalInput")
    out_d = nc.dram_tensor("out", [O, 1], F32, kind="ExternalOutput")

    with tile.TileContext(nc) as tc:
        with (
            tc.tile_pool(name="const", bufs=1) as const,
            tc.tile_pool(name="act", bufs=1) as act,
            tc.tile_pool(name="tmp", bufs=3) as tmp,
            tc.tile_pool(name="rows", bufs=1) as rowsp,
            tc.tile_pool(name="pseg", bufs=2, space="PSUM") as pseg,
            tc.tile_pool(name="prow", bufs=2, space="PSUM") as prow,
            tc.tile_pool(name="pf", bufs=1, space="PSUM") as pfp,
            tc.tile_pool(name="pt", bufs=2, space="PSUM") as pt,
        ):
            # ---- DMA issue order: mask/cnt, hidden (all 4 options), wp,
            # wa0/wa, biases, wl0 (in classify-chunk consumption order).
            mask_sb = const.tile([P, T, NK], HDT)
            nc.sync.dma_start(out=mask_sb, in_=mask_d[:])
            cnt_sb = const.tile([NK, 1], F32)
            nc.sync.dma_start(out=cnt_sb, in_=cnt_d[:])

            # hidden: l = p*T + t layout -> per-partition contiguous reads
            hid_r = hid_d.rearrange("o (p t) e -> o p t e", t=T)
            htile = const.tile([P, O, T, E], HDT)
            for o in range(O):
                for h in range(2):
                    nc.sync.dma_start(
                        out=htile[:, o, h * 4 : h * 4 + 4, :],
                        in_=hid_r[o][:, h * 4 : h * 4 + 4, :],
                    )
            wp_sb = const.tile([P, 8, 1024], BF16)
            nc.sync.dma_start(out=wp_sb[:, 0:4, :], in_=wp_d[:, 0:4, :])
            nc.sync.dma_start(out=wp_sb[:, 4:8, :], in_=wp_d[:, 4:8, :])
            wa0_sb = const.tile([P, 8, 512], BF16)
            nc.sync.dma_start(out=wa0_sb, in_=wa0_d[:])
            wa_sb = const.tile([P, 4, 512], BF16)
            nc.sync.dma_start(out=wa_sb, in_=wa_d[:])
            bias_sb = const.tile([P, 21], F32)
            nc.sync.dma_start(out=bias_sb, in_=bias_d[:])
            bl0r_sb = const.tile([O, 512], F32)
            nc.sync.dma_start(out=bl0r_sb, in_=bl0r_d[:])
            wlr_sb = const.tile([O, 512], F32)
            nc.sync.dma_start(out=wlr_sb, in_=wlr_d[:])
            wl0_sb = const.tile([P, 32, 512], BF16)
            for a, b in ((8, 16), (16, 24), (24, 32), (0, 8)):
                nc.sync.dma_start(out=wl0_sb[:, a:b, :], in_=wl0_d[:, a:b, :])

            ident = const.tile([P, P], F32)
            make_identity(nc, ident)

            def bcol(i):
                return bias_sb[:, i : i + 1]

            # ---- segment sums: ps[k, e] = sum over rows of seg k (0/1 mask)
            # then x = ps * cntinv; x_all packs one 32-aligned row block per
            # option (partition bases must be 32-aligned)
            x_all = rowsp.tile([P, E], F32, tag="x_all")
            nc.vector.memset(x_all, 0.0)
            for o in range(O):
                for half in range(2):
                    sl = slice(half * 512, half * 512 + 512)
                    ps = pseg.tile([NK, 512], F32, tag="ps_seg")
                    for t in range(T):
                        nc.tensor.matmul(
                            out=ps,
                            lhsT=mask_sb[:, t, :],
                            rhs=htile[:, o, t, sl],
                            start=(t == 0),
                            stop=(t == T - 1),
                        )
                    nc.vector.tensor_scalar_mul(
                        out=x_all[o * 32 : o * 32 + NK, sl],
                        in0=ps[:, :],
                        scalar1=cnt_sb[:, :],
                    )
            # transpose to xT[c, o, k] (bf16), 128 columns at a time
            xT = act.tile([P, 8, O, NK], BF16)
            for c in range(8):
                ptile = pt.tile([P, P], F32, tag="pt")
                nc.tensor.transpose(
                    out=ptile,
                    in_=x_all[:, c * P : (c + 1) * P],
                    identity=ident[:, :],
                )
                nc.scalar.copy(
                    out=xT[:, c, :, :],
                    in_=ptile.rearrange("p (o k) -> p o k", k=32)[:, :, 0:NK],
                )

            def flip_layer(
                name,
                lhs_chunks,  # list of bf16 [P, R] stationary APs (K chunks)
                w_sb,  # weight tile, [P, K/128, NW] layout
                n_out,  # output features
                r,  # rows (= lhs free size)
            ):
                """out rows = (lhs^T)^T @ W, returns the row-major sbuf copy
                [r, n_out] (per-512 psum chunks copied out)."""
                rows_full = rowsp.tile([NCOL, 1024], F32, tag="rows_sh")
                rows_sb = rows_full[:r, :n_out]
                for n2 in range(n_out // 512):
                    pr = prow.tile([r, 512], F32, tag="prow")
                    for c, lhs in enumerate(lhs_chunks):
                        nc.tensor.matmul(
                            out=pr,
                            lhsT=lhs,
                            rhs=w_sb[:, c, n2 * 512 : (n2 + 1) * 512]
                            if w_sb.shape[2] > 512
                            else w_sb[:, c, :],
                            start=(c == 0),
                            stop=(c == len(lhs_chunks) - 1),
                        )
                    nc.scalar.copy(
                        out=rows_sb[:, n2 * 512 : (n2 + 1) * 512], in_=pr[:, :]
                    )
                return rows_sb

            def transpose_rows(rows_sb, r, n_out):
                """Yield (mc, psum [P, r]) transposed feature chunks."""
                for mc in range(n_out // P):
                    ptile = pt.tile([P, r], F32, tag="pt")
                    nc.tensor.transpose(
                        out=ptile,
                        in_=rows_sb[:, mc * P : (mc + 1) * P],
                        identity=ident[:r, :r],
                    )
                    yield mc, ptile

            # ---- projection: e = max(x @ Wp + (bp + 1), 1)
            eT = act.tile([P, 8, O, NK], F32)
            eTb = act.tile([P, 8, NCOL], BF16)
            xT_chunks = [xT[:, c, :, :] for c in range(8)]
            rows_e = flip_layer("e", xT_chunks, wp_sb, 1024, NCOL)
            for mc, ptile in transpose_rows(rows_e, NCOL, 1024):
                nc.vector.tensor_scalar(
                    out=eT[:, mc, :, :],
                    in0=ptile[:, :],
                    scalar1=bcol(mc),
                    scalar2=1.0,
                    op0=OP.add,
                    op1=OP.max,
                )
                nc.vector.tensor_copy(out=eTb[:, mc, :], in_=eT[:, mc, :, :])

            # catF chunks 8..31 (a_ac, b_ac, a_o, b_o, a_q, b_q) only need eT;
            # filling them now lets the classify-head matmuls over those
            # chunks run inside tensor-engine gaps during the softmax phases.
            catF = act.tile([P, 32, O], F32)
            for j, (half, k) in enumerate(
                ((0, 12), (1, 12), (0, 11), (1, 11), (0, 10), (1, 10))
            ):
                nc.gpsimd.tensor_copy(
                    out=catF[:, 8 + j * 4 : 12 + j * 4, :],
                    in_=eT[:, half * 4 : half * 4 + 4, :, k],
                )
            catFb = act.tile([P, 32, O], BF16)
            for j in range(3):
                nc.gpsimd.tensor_copy(
                    out=catFb[:, 8 + j * 8 : 16 + j * 8, :],
                    in_=catF[:, 8 + j * 8 : 16 + j * 8, :],
                )

            # ---- pool 1 (intersection): h1 = relu(e @ Wa0 + ba0) (bf16 out)
            h1Tb = act.tile([P, 4, NCOL], BF16)
            rows_h1 = flip_layer(
                "h1", [eTb[:, c, :] for c in range(8)], wa0_sb, 512, NCOL
            )
            for mc, ptile in transpose_rows(rows_h1, NCOL, 512):
                nc.vector.tensor_scalar(
                    out=h1Tb[:, mc, :],
                    in0=ptile[:, :],
                    scalar1=bcol(8 + mc),
                    scalar2=0.0,
                    op0=OP.add,
                    op1=OP.max,
                )

            # l1 = h1 @ Wa + ba (fp32, shared by pool 1 softmax and renew)
            l1T = act.tile([P, 4, O, NK], F32)
            rows_l1 = flip_layer(
                "l1", [h1Tb[:, c, :] for c in range(4)], wa_sb, 512, NCOL
            )
            for mc, ptile in transpose_rows(rows_l1, NCOL, 512):
                nc.vector.tensor_scalar_add(
                    out=l1T[:, mc, :, :], in0=ptile[:, :], scalar1=bcol(12 + mc)
                )

            # ---- classify head part 1: chunks 8..19 into the pf PSUM bank
            # (only need catFb + wl0; they run while the vector engine does
            # the pool-1 softmax below)
            pf = pfp.tile([O, 512], F32, tag="pf")
            kc_order = list(range(8, 32)) + list(range(8))
            for i, kc in enumerate(kc_order[:12]):
                nc.tensor.matmul(
                    out=pf,
                    lhsT=catFb[:, kc, :],
                    rhs=wl0_sb[:, kc, :],
                    start=(i == 0),
                    stop=False,
                )

            # pool 1 softmax over the 10 ctx segments + weighted reduce
            # (no max-subtraction: logits are O(1); batched [P, 4, O, 10])
            cat2 = act.tile([P, 8, O], F32)
            cat2b = act.tile([P, 8, O], BF16)
            lsl = l1T[:, :, :, 0:10]
            w = tmp.tile([P, 4, O, 10], F32, tag="w")
            nc.scalar.activation(out=w, in_=lsl, func=AF.Exp)
            s = tmp.tile([P, 4, O], F32, tag="s")
            nc.vector.reduce_sum(s, w, axis=AX)
            r = tmp.tile([P, 4, O], F32, tag="r")
            nc.vector.reciprocal(out=r, in_=s)
            for half in range(2):
                wt = tmp.tile([P, 4, O, 10], F32, tag="wt")
                nc.vector.tensor_tensor(
                    out=wt, in0=w, in1=eT[:, half * 4 : half * 4 + 4, :, 0:10],
                    op=OP.mult,
                )
                st = tmp.tile([P, 4, O], F32, tag="st")
                nc.vector.reduce_sum(st, wt, axis=AX)
                nc.vector.tensor_tensor(
                    out=cat2[:, half * 4 : half * 4 + 4, :], in0=st, in1=r,
                    op=OP.mult,
                )
            nc.vector.tensor_copy(out=cat2b, in_=cat2)

            # ---- renew: h2/l2 for the intersection pair element
            h2Tb = act.tile([P, 4, O], BF16)
            rows_h2 = flip_layer(
                "h2", [cat2b[:, c, :] for c in range(8)], wa0_sb, 512, O
            )
            for mc, ptile in transpose_rows(rows_h2, O, 512):
                nc.vector.tensor_scalar(
                    out=h2Tb[:, mc, :],
                    in0=ptile[:, :],
                    scalar1=bcol(8 + mc),
                    scalar2=0.0,
                    op0=OP.add,
                    op1=OP.max,
                )
            l2T = act.tile([P, 4, O], F32)
            rows_l2 = flip_layer(
                "l2", [h2Tb[:, c, :] for c in range(4)], wa_sb, 512, O
            )
            for mc, ptile in transpose_rows(rows_l2, O, 512):
                nc.vector.tensor_scalar_add(
                    out=l2T[:, mc, :], in0=ptile[:, :], scalar1=bcol(12 + mc)
                )

            # ---- classify head part 2: chunks 20..31 (run under the pair
            # softmax)
            for kc in kc_order[12:24]:
                nc.tensor.matmul(
                    out=pf,
                    lhsT=catFb[:, kc, :],
                    rhs=wl0_sb[:, kc, :],
                    start=False,
                    stop=False,
                )

            # pair softmax([l1[k], l2]) -> na/nb; store reciprocals
            # (no max-subtraction; e2/t2 computed at [P, 4, O] then broadcast)
            raT = act.tile([P, 4, O, 10], F32)
            rbT = act.tile([P, 4, O, 10], F32)
            raTb = act.tile([P, 4, O, 10], BF16)
            rbTb = act.tile([P, 4, O, 10], BF16)
            l1s = l1T[:, :, :, 0:10]
            e1 = tmp.tile([P, 4, O, 10], F32, tag="e1")
            nc.scalar.activation(out=e1, in_=l1s, func=AF.Exp)
            e2 = tmp.tile([P, 4, O], F32, tag="e2")
            nc.scalar.activation(out=e2, in_=l2T[:, :, :], func=AF.Exp)
            s12 = tmp.tile([P, 4, O, 10], F32, tag="s12")
            nc.vector.tensor_tensor(
                out=s12, in0=e1, in1=e2.broadcast_to([P, 4, O, 10]), op=OP.add
            )
            rs = tmp.tile([P, 4, O, 10], F32, tag="rs")
            nc.vector.reciprocal(out=rs, in_=s12)
            for half, dst, dstb in ((0, raT, raTb), (1, rbT, rbTb)):
                t1 = tmp.tile([P, 4, O, 10], F32, tag="t1")
                nc.vector.tensor_tensor(
                    out=t1,
                    in0=e1,
                    in1=eT[:, half * 4 : half * 4 + 4, :, 0:10],
                    op=OP.mult,
                )
                t2 = tmp.tile([P, 4, O], F32, tag="t2")
                nc.vector.tensor_tensor(
                    out=t2, in0=e2, in1=cat2[:, half * 4 : half * 4 + 4, :],
                    op=OP.mult,
                )
                t3 = tmp.tile([P, 4, O, 10], F32, tag="t3")
                nc.vector.tensor_tensor(
                    out=t3, in0=t1, in1=t2.broadcast_to([P, 4, O, 10]),
                    op=OP.add,
                )
                nv = tmp.tile([P, 4, O, 10], F32, tag="nv")
                nc.vector.tensor_tensor(out=nv, in0=t3, in1=rs, op=OP.mult)
                nc.vector.reciprocal(out=dst[:, :, :, :], in_=nv)
                nc.vector.tensor_copy(out=dstb[:, :, :, :], in_=dst[:, :, :, :])

            # ---- union pool over segments of [1/na; 1/nb]
            h3Tb = act.tile([P, 4, O, 10], BF16)
            rows_h3 = flip_layer(
                "h3",
                [raTb[:, c, :, :] for c in range(4)]
                + [rbTb[:, c, :, :] for c in range(4)],
                wa0_sb,
                512,
                O * 10,
            )
            for mc, ptile in transpose_rows(rows_h3, O * 10, 512):
                nc.vector.tensor_scalar(
                    out=h3Tb[:, mc, :, :],
                    in0=ptile[:, :],
                    scalar1=bcol(8 + mc),
                    scalar2=0.0,
                    op0=OP.add,
                    op1=OP.max,
                )
            l3T = act.tile([P, 4, O, 10], F32)
            rows_l3 = flip_layer(
                "l3", [h3Tb[:, c, :, :] for c in range(4)], wa_sb, 512, O * 10
            )
            for mc, ptile in transpose_rows(rows_l3, O * 10, 512):
                nc.vector.tensor_scalar_add(
                    out=l3T[:, mc, :, :], in0=ptile[:, :], scalar1=bcol(12 + mc)
                )

            # union softmax + weighted reduce + invert -> catF chunks 0..7
            # ua = recip((sum_k w3 ra) * recip(sum_k w3))
            w3 = tmp.tile([P, 4, O, 10], F32, tag="w3")
            nc.scalar.activation(out=w3, in_=l3T[:, :, :, :], func=AF.Exp)
            s3 = tmp.tile([P, 4, O], F32, tag="s3")
            nc.vector.reduce_sum(s3, w3, axis=AX)
            r3 = tmp.tile([P, 4, O], F32, tag="r3")
            nc.vector.reciprocal(out=r3, in_=s3)
            for half, src in ((0, raT), (1, rbT)):
                tu = tmp.tile([P, 4, O, 10], F32, tag="tu")
                nc.vector.tensor_tensor(
                    out=tu, in0=w3, in1=src[:, :, :, :], op=OP.mult
                )
                su = tmp.tile([P, 4, O], F32, tag="su")
                nc.vector.reduce_sum(su, tu, axis=AX)
                nu = tmp.tile([P, 4, O], F32, tag="nu")
                nc.vector.tensor_tensor(out=nu, in0=su, in1=r3, op=OP.mult)
                nc.vector.reciprocal(
                    out=catF[:, half * 4 : half * 4 + 4, :], in_=nu
                )
            nc.vector.tensor_copy(
                out=catFb[:, 0:8, :], in_=catF[:, 0:8, :]
            )

            # ---- classify head part 3: chunks 0..7 (ua/ub), close the
            # accumulation
            for i, kc in enumerate(kc_order[24:]):
                nc.tensor.matmul(
                    out=pf,
                    lhsT=catFb[:, kc, :],
                    rhs=wl0_sb[:, kc, :],
                    start=False,
                    stop=(i == 7),
                )
            # out = relu(hf + bl0) . Wl + bl, all on the vector engine
            hrelu = rowsp.tile([O, 512], F32, tag="hrelu")
            nc.vector.tensor_tensor(out=hrelu, in0=pf[:, :], in1=bl0r_sb, op=OP.add)
            nc.vector.tensor_scalar_max(out=hrelu, in0=hrelu, scalar1=0.0)
            hw = rowsp.tile([O, 512], F32, tag="hw")
            nc.vector.tensor_tensor(out=hw, in0=hrelu, in1=wlr_sb, op=OP.mult)
            osum = rowsp.tile([O, 1], F32, tag="osum")
            nc.vector.reduce_sum(osum, hw, axis=AX)
            out_sb = rowsp.tile([O, 1], F32, tag="out_sb")
            nc.vector.tensor_scalar_add(
                out=out_sb, in0=osum, scalar1=bias_sb[0:O, 20:21]
            )
            nc.sync.dma_start(out=out_d[:], in_=out_sb)

            if debug:
                for name, t in (
                    ("xT", xT),
                    ("eT", eT),
                    ("l1T", l1T),
                    ("cat2", cat2),
                    ("raT", raT),
                    ("rbT", rbT),
                    ("catF", catF),
                ):
                    dt = F32 if t is not xT else BF16
                    d = nc.dram_tensor(
                        "dbg_" + name, list(t.shape), dt, kind="ExternalOutput"
                    )
                    nc.sync.dma_start(out=d[:], in_=t)

    _split_excess_waits(nc)
    return nc


_NC = None


def _get_nc():
    global _NC
    if _NC is None:
        _NC = _build_nc()
    return _NC


def _prep_inputs(hidden, idx, Wp, bp, Wa0, ba0, Wa, ba, Wl0, bl0, Wl, bl):
    hidden = np.asarray(hidden, dtype=np.float32)
    idx = np.asarray(idx).astype(np.int64)

    f32 = lambda a: np.ascontiguousarray(np.asarray(a, dtype=np.float32))
    bf = lambda a: np.ascontiguousarray(np.asarray(a, dtype=np.float32).astype(NPBF16))
    bp, ba0, ba, bl0, bl = f32(bp), f32(ba0), f32(ba), f32(bl0), f32(bl)
    Wl = f32(Wl)

    hid_b = np.ascontiguousarray(hidden.astype(NPHDT))  # [B, O, L, E]
    wp_t = bf(np.asarray(Wp, np.float32).reshape(8, P, 1024).transpose(1, 0, 2))
    wa0_t = bf(np.asarray(Wa0, np.float32).reshape(8, P, 512).transpose(1, 0, 2))
    wa_t = bf(np.asarray(Wa, np.float32).reshape(4, P, 512).transpose(1, 0, 2))
    wl0_t = bf(np.asarray(Wl0, np.float32).reshape(32, P, 512).transpose(1, 0, 2))

    biases = np.zeros((P, 21), dtype=np.float32)
    biases[:, 0:8] = (bp + 1.0).reshape(8, P).T
    biases[:, 8:12] = ba0.reshape(4, P).T
    biases[:, 12:16] = ba.reshape(4, P).T
    biases[:, 16:20] = bl0.reshape(4, P).T
    biases[:, 20] = bl[0]

    bl0rep = np.ascontiguousarray(np.broadcast_to(bl0, (O, 512)).astype(np.float32))
    wlrep = np.ascontiguousarray(np.broadcast_to(Wl[:, 0], (O, 512)).astype(np.float32))

    in_maps = []
    for b in range(B):
        m = np.zeros((L, NK), dtype=np.float32)
        cntinv = np.zeros((NK, 1), dtype=np.float32)
        ib = idx[b]
        starts = [1] + [int(ib[k]) for k in range(9)]
        ends = [int(ib[k]) for k in range(10)]
        segs = [(starts[k], ends[k]) for k in range(10)]
        segs.append((int(ib[9]), int(ib[10])))
        segs.append((int(ib[10]), int(ib[11])))
        segs.append((1, int(ib[9])))
        for k, (s, e) in enumerate(segs):
            m[s:e, k] = 1.0
            cntinv[k, 0] = 1.0 / (e - s)
        # l = p*T + t layout (matches the hidden SBUF tiling)
        maskt = np.ascontiguousarray(m.reshape(P, T, NK).astype(NPHDT))

        in_maps.append(
            dict(
                hidden=np.ascontiguousarray(hid_b[b]),
                maskt=maskt,
                cntinv=cntinv,
                wp=wp_t,
                wa0=wa0_t,
                wa=wa_t,
                wl0=wl0_t,
                biases=biases,
                bl0rep=bl0rep,
                wlrep=wlrep,
            )
        )
    return in_maps


def _run(in_maps, **kwargs):
    return run_bass_kernel_spmd(_get_nc(), in_maps, core_ids=list(range(B)), **kwargs)


def kernel(**inputs):
    in_maps = _prep_inputs(**inputs)
    res = _run(in_maps)
    return np.stack([r["out"].reshape(O, 1) for r in res.results])


def _install_ntff_hook():
    """The RL container's antenv lacks axon_hooks, so boot() skipped NTFF
    hook registration. Recreate the module and register the ctypes hook."""
    import sys
    import types

    name = "antenv.axon_hooks"
    if name not in sys.modules:
        try:
            __import__(name)
        except ImportError:
            mod = types.ModuleType(name)
            mod._hook = None
            mod.set_axon_ntff_profile_hook = lambda h: setattr(mod, "_hook", h)
            mod.get_axon_ntff_profile_hook = lambda: mod._hook
            sys.modules[name] = mod
            import antenv

            antenv.axon_hooks = mod
    import antenv.axon_hooks as ah

    if ah.get_axon_ntff_profile_hook() is None:
        from trn_agent_boot.trn_boot import _ntff_profile_via_ctypes

        ah.set_axon_ntff_profile_hook(
            _ntff_profile_via_ctypes("/opt/axon/libaxon_pjrt.so")
        )

    import concourse.bass_utils as bu

    bu.upload_artifacts = lambda tmpdir: tmpdir


def benchmark(trace_cores=None, **inputs):
    """Run with NTFF tracing; returns (output, BassKernelResults)."""
    _install_ntff_hook()
    in_maps = _prep_inputs(**inputs)
    res = _run(in_maps, trace=True, trace_cores=trace_cores)
    out = np.stack([r["out"].reshape(O, 1) for r in res.results])
    return out, res


# revision 6
# speedup vs baseline: 1.8157x; 1.1232x over previous
"""Trainium2 Bass kernel for nn_Beta_LR_41308995453190.

Network (per (b, o) pair):
  - 13 segment means over the L axis of hidden[b, o] (ragged boundaries
    from idx[b]): 10 context segments, question, option, whole-context.
  - beta-param projection e = 1 + relu(x @ Wp + bp), split a/b.
  - three attention pools (intersection over segments, renew over
    (segment, intersection) pairs, union over inverted renewed params).
  - classify head: concat 8 beta embeddings -> relu(@Wl0 + bl0) -> @Wl + bl.

Sharding: data-parallel over the batch dim B=8 (one batch per NeuronCore),
weights replicated.

Implementation notes:
  - Segment sums are 0/1-mask matmuls (mask as the 13-column stationary
    operand, hidden streaming 512 wide), scaled by 1/count afterwards.
    Hidden and mask travel in fp8 e3m4 (the mask is exactly representable,
    products accumulate in fp32 PSUM), halving the dominant DMA stream;
    hidden is laid out l = p*T + t so each partition's read is one
    contiguous 8KB block, and all four options are SBUF-resident so the
    segsum phase never stalls on buffer reuse.
  - The beta projection runs "flipped" (xT stationary, Wp streaming 512
    wide); its PSUM rows are copied out in 128-column pieces so the
    feature transposes pipeline behind the copies.
  - Everything downstream (h1/l1, h2/l2, h3/l3) runs weight-stationary
    and feature-major: the Wa0/Wa 128x128 blocks are the stationary
    operand and the (tiny) activation blocks stream.  No transposes, no
    PSUM->SBUF row copies; the per-feature bias and nonlinearity are
    fused into one scalar-engine activation (relu / exp) that reads the
    PSUM output directly.  The softmaxes skip max-subtraction (logits
    are 0.02-scale products, bounded far inside exp range) and use the
    fast approximate reciprocal.
  - The classify head streams bf16 Wl0 against the catF column blocks
    into a dedicated PSUM bank in three emission groups interleaved
    with the pool phases, so those matmuls fill the PE gaps under the
    softmax chains; bl0 is added by one extra accumulation matmul
    (ones-column x bl0/128), and the final relu/dot/+bl collapses into
    a tensor_scalar_max plus one fused tensor_tensor_reduce.
"""

import numpy as np
import ml_dtypes

try:
    import concourse.bass as bass
except ImportError:
    import sys

    sys.path.insert(0, "/opt/trn_rl_repo")
    import concourse.bass as bass

import concourse.tile as tile
from concourse import mybir
from concourse.bass_utils import run_bass_kernel_spmd
from concourse.masks import make_identity

F32 = mybir.dt.float32
BF16 = mybir.dt.bfloat16
F8 = mybir.dt.float8e3
NPBF16 = ml_dtypes.bfloat16
NPF8 = ml_dtypes.float8_e3m4
AX = mybir.AxisListType.X
OP = mybir.AluOpType
AF = mybir.ActivationFunctionType

B, O, L, E = 8, 4, 1024, 1024
BETA = 512
NSEG = 12
NK = 13  # 10 ctx + q + o + allc
P = 128
T = L // P  # 8 L-tiles per option
NCOL = O * NK  # 52

HID_FP8 = True  # hidden + mask in fp8 e3m4 (halves hidden DMA)

HDT, NPHDT = (F8, NPF8) if HID_FP8 else (BF16, NPBF16)


def _split_excess_waits(nc, max_waits=1):
    """This neuronxcc walrus build rejects more than one sem wait per TPB
    instruction; hoist excess waits onto drain instructions inserted before
    the offending instruction on the same engine."""
    scratch_bb = nc.cur_bb.bb
    for f in nc.m.functions:
        for bb in f.blocks:
            new_list = []
            for ins in bb.instructions:
                si = ins.sync_info
                waits = list(si.on_wait) if si and si.on_wait else []
                if len(waits) > max_waits:
                    for w in waits[: len(waits) - max_waits]:
                        carrier = nc.engines[ins.engine].nop(nofuse=True).ins
                        scratch_bb.instructions.remove(carrier)
                        carrier.sync_info = mybir.SyncInfo(
                            on_wait=[w], on_update=[]
                        )
                        new_list.append(carrier)
                    si.on_wait = waits[len(waits) - max_waits :]
                new_list.append(ins)
            bb.instructions[:] = new_list


def _build_nc(debug=False):
    nc = bass.Bass("TRN2", target_bir_lowering=False)

    hid_d = nc.dram_tensor("hidden", [O, L, E], HDT, kind="ExternalInput")
    mask_d = nc.dram_tensor("maskt", [P, T, NK], HDT, kind="ExternalInput")
    cnt_d = nc.dram_tensor("cntinv", [NK, 1], F32, kind="ExternalInput")
    wp_d = nc.dram_tensor("wp", [P, 8, 1024], BF16, kind="ExternalInput")
    wa0_d = nc.dram_tensor("wa0", [P, 8, 512], BF16, kind="ExternalInput")
    wa_d = nc.dram_tensor("wa", [P, 4, 512], BF16, kind="ExternalInput")
    wl0_d = nc.dram_tensor("wl0", [P, 32, 512], BF16, kind="ExternalInput")
    bias_d = nc.dram_tensor("biases", [P, 21], F32, kind="ExternalInput")
    bl0r_d = nc.dram_tensor("bl0rep", [P, 512], BF16, kind="ExternalInput")
    wlr_d = nc.dram_tensor("wlrep", [O, 512], F32, kind="ExternalInput")
    out_d = nc.dram_tensor("out", [O, 1], F32, kind="ExternalOutput")

    with tile.TileContext(nc) as tc:
        with (
            tc.tile_pool(name="const", bufs=1) as const,
            tc.tile_pool(name="act", bufs=1) as act,
            tc.tile_pool(name="tmp", bufs=3) as tmp,
            tc.tile_pool(name="rows", bufs=1) as rowsp,
            tc.tile_pool(name="pseg", bufs=2, space="PSUM") as pseg,
            tc.tile_pool(name="pwork", bufs=2, space="PSUM") as pwork,
            tc.tile_pool(name="pf", bufs=1, space="PSUM") as pfp,
            tc.tile_pool(name="pt", bufs=2, space="PSUM") as pt,
        ):
            # ---- DMA issue order: mask, hidden o0 (fine-grained), cnt,
            # hidden o1-3, wp, wa0/wa, biases, wl0 (classify-chunk order).
            mask_sb = const.tile([P, T, NK], HDT)
            nc.sync.dma_start(out=mask_sb, in_=mask_d[:])

            # hidden: l = p*T + t layout -> per-partition contiguous reads
            hid_r = hid_d.rearrange("o (p t) e -> o p t e", t=T)
            htile = const.tile([P, O, T, E], HDT)
            for h in range(4):
                nc.sync.dma_start(
                    out=htile[:, 0, h * 2 : h * 2 + 2, :],
                    in_=hid_r[0][:, h * 2 : h * 2 + 2, :],
                )
            cnt_sb = const.tile([NK, 1], F32)
            nc.sync.dma_start(out=cnt_sb, in_=cnt_d[:])
            for o in range(1, O):
                for h in range(2):
                    nc.sync.dma_start(
                        out=htile[:, o, h * 4 : h * 4 + 4, :],
                        in_=hid_r[o][:, h * 4 : h * 4 + 4, :],
                    )
            wp_sb = const.tile([P, 8, 1024], BF16)
            nc.sync.dma_start(out=wp_sb[:, 0:4, :], in_=wp_d[:, 0:4, :])
            nc.sync.dma_start(out=wp_sb[:, 4:8, :], in_=wp_d[:, 4:8, :])
            wa0_sb = const.tile([P, 8, 512], BF16)
            nc.sync.dma_start(out=wa0_sb, in_=wa0_d[:])
            wa_sb = const.tile([P, 4, 512], BF16)
            nc.sync.dma_start(out=wa_sb, in_=wa_d[:])
            bias_sb = const.tile([P, 21], F32)
            nc.sync.dma_start(out=bias_sb, in_=bias_d[:])
            bl0r_sb = const.tile([P, 512], BF16)
            nc.sync.dma_start(out=bl0r_sb, in_=bl0r_d[:])
            wlr_sb = const.tile([O, 512], F32)
            nc.sync.dma_start(out=wlr_sb, in_=wlr_d[:])
            wl0_sb = const.tile([P, 32, 512], BF16)
            for a, b in ((8, 16), (16, 24), (24, 32), (0, 8)):
                nc.sync.dma_start(out=wl0_sb[:, a:b, :], in_=wl0_d[:, a:b, :])

            ident = const.tile([P, P], F32)
            make_identity(nc, ident)
            ones4 = const.tile([P, O], BF16)
            nc.vector.memset(ones4, 1.0)

            def bcol(i):
                return bias_sb[:, i : i + 1]

            # ---- segment sums: ps[k, e] = sum over rows of seg k (0/1 mask)
            # then x = ps * cntinv; x_all packs one 32-aligned row block per
            # option (partition bases must be 32-aligned)
            x_all = rowsp.tile([P, E], F32, tag="x_all")
            nc.vector.memset(x_all, 0.0)
            for o in range(O):
                for half in range(2):
                    sl = slice(half * 512, half * 512 + 512)
                    ps = pseg.tile([NK, 512], F32, tag="ps_seg")
                    for t in range(T):
                        nc.tensor.matmul(
                            out=ps,
                            lhsT=mask_sb[:, t, :],
                            rhs=htile[:, o, t, sl],
                            start=(t == 0),
                            stop=(t == T - 1),
                        )
                    nc.vector.tensor_scalar_mul(
                        out=x_all[o * 32 : o * 32 + NK, sl],
                        in0=ps[:, :],
                        scalar1=cnt_sb[:, :],
                    )
            # transpose to xT[c, o, k] (bf16), 128 columns at a time
            xT = act.tile([P, 8, O, NK], BF16)
            for c in range(8):
                ptile = pt.tile([P, P], F32, tag="pt")
                nc.tensor.transpose(
                    out=ptile,
                    in_=x_all[:, c * P : (c + 1) * P],
                    identity=ident[:, :],
                )
                nc.scalar.copy(
                    out=xT[:, c, :, :],
                    in_=ptile.rearrange("p (o k) -> p o k", k=32)[:, :, 0:NK],
                )

            # ---- projection (flipped): eTb[f, (o,k)] = max(x @ Wp + bp', 1)
            # PSUM rows copied out 128 columns at a time so the transposes
            # pipeline behind the copies.
            eTb = act.tile([P, 8, NCOL], BF16)
            rows_e = rowsp.tile([NCOL, 1024], F32, tag="rows_sh")
            for n2 in range(2):
                pr = pwork.tile([NCOL, 512], F32, tag="pwork")
                for c in range(8):
                    nc.tensor.matmul(
                        out=pr,
                        lhsT=xT[:, c, :, :],
                        rhs=wp_sb[:, c, n2 * 512 : (n2 + 1) * 512],
                        start=(c == 0),
                        stop=(c == 7),
                    )
                for i in range(4):
                    mc = n2 * 4 + i
                    nc.vector.tensor_copy(
                        out=rows_e[:, mc * P : (mc + 1) * P],
                        in_=pr[:, i * P : (i + 1) * P],
                    )
            for mc in range(8):
                ptile = pt.tile([P, NCOL], F32, tag="pt")
                nc.tensor.transpose(
                    out=ptile,
                    in_=rows_e[:, mc * P : (mc + 1) * P],
                    identity=ident[:NCOL, :NCOL],
                )
                nc.vector.tensor_scalar(
                    out=eTb[:, mc, :],
                    in0=ptile[:, :],
                    scalar1=bcol(mc),
                    scalar2=1.0,
                    op0=OP.add,
                    op1=OP.max,
                )
            eTr = eTb.rearrange("p c (o k) -> p c o k", k=NK)

            # catFb chunks 8..31 (a_ac, b_ac, a_o, b_o, a_q, b_q) only need
            # eTb; filling them now lets the classify-head matmuls over those
            # chunks run inside tensor-engine gaps during the softmax phases.
            catFb = act.tile([P, 32, O], BF16)
            for j, (half, k) in enumerate(
                ((0, 12), (1, 12), (0, 11), (1, 11), (0, 10), (1, 10))
            ):
                nc.gpsimd.tensor_copy(
                    out=catFb[:, 8 + j * 4 : 12 + j * 4, :],
                    in_=eTr[:, half * 4 : half * 4 + 4, :, k],
                )

            # ---- pool 1 (intersection), weight-stationary feature-major:
            # h1 = relu(e @ Wa0 + ba0), only the 40 ctx rows
            h1Tb = act.tile([P, 4, O, 10], BF16)
            h1ps = pwork.tile([P, 4, O * 10], F32, tag="pwork", space="PSUM")
            for f in range(4):
                for c in range(8):
                    nc.tensor.matmul(
                        out=h1ps[:, f, :],
                        lhsT=wa0_sb[:, c, f * P : (f + 1) * P],
                        rhs=eTr[:, c, :, 0:10],
                        start=(c == 0),
                        stop=(c == 7),
                    )
                nc.scalar.activation(
                    out=h1Tb[:, f, :, :],
                    in_=h1ps.rearrange("p f (o k) -> p f o k", o=O)[:, f],
                    func=AF.Relu,
                    bias=bcol(8 + f),
                )

            # l1 = h1 @ Wa + ba; w = exp(l1) fused from PSUM (reused as the
            # pair-softmax numerator e1 below)
            w = act.tile([P, 4, O, 10], F32)
            l1ps = pwork.tile([P, 4, O * 10], F32, tag="pwork", space="PSUM")
            for f in range(4):
                for c in range(4):
                    nc.tensor.matmul(
                        out=l1ps[:, f, :],
                        lhsT=wa_sb[:, c, f * P : (f + 1) * P],
                        rhs=h1Tb[:, c, :, :],
                        start=(c == 0),
                        stop=(c == 3),
                    )
                nc.scalar.activation(
                    out=w[:, f, :, :],
                    in_=l1ps.rearrange("p f (o k) -> p f o k", o=O)[:, f],
                    func=AF.Exp,
                    bias=bcol(12 + f),
                )

            # ---- classify head part 1: bl0 (via ones x bl0/128) + chunks
            # 8..19; these run on the PE while the vector engine does the
            # pool-1 softmax below.
            pf = pfp.tile([O, 512], F32, tag="pf")
            nc.tensor.matmul(
                out=pf, lhsT=ones4, rhs=bl0r_sb, start=True, stop=False
            )
            kc_order = list(range(8, 32)) + list(range(8))
            for kc in kc_order[:12]:
                nc.tensor.matmul(
                    out=pf,
                    lhsT=catFb[:, kc, :],
                    rhs=wl0_sb[:, kc, :],
                    start=False,
                    stop=False,
                )

            # pool 1 softmax over the 10 ctx segments + weighted reduce
            # (no max-subtraction; wt_a/wt_b are reused by the pair phase)
            cat2b = act.tile([P, 8, O], BF16)
            s = tmp.tile([P, 4, O], F32, tag="s")
            nc.vector.reduce_sum(s, w, axis=AX)
            r = tmp.tile([P, 4, O], F32, tag="r")
            nc.vector.reciprocal(out=r, in_=s)
            wts = []
            for half in range(2):
                wt = tmp.tile([P, 4, O, 10], F32, tag=f"wt{half}")
                nc.vector.tensor_tensor(
                    out=wt, in0=w, in1=eTr[:, half * 4 : half * 4 + 4, :, 0:10],
                    op=OP.mult,
                )
                wts.append(wt)
                st = tmp.tile([P, 4, O], F32, tag="st")
                nc.vector.reduce_sum(st, wt, axis=AX)
                nc.vector.tensor_tensor(
                    out=cat2b[:, half * 4 : half * 4 + 4, :], in0=st, in1=r,
                    op=OP.mult,
                )

            # ---- renew: h2/l2 for the intersection pair element
            # (weight-stationary, 4 rows)
            h2Tb = act.tile([P, 4, O], BF16)
            h2ps = pwork.tile([P, 4, O], F32, tag="pwork", space="PSUM")
            for f in range(4):
                for c in range(8):
                    nc.tensor.matmul(
                        out=h2ps[:, f, :],
                        lhsT=wa0_sb[:, c, f * P : (f + 1) * P],
                        rhs=cat2b[:, c, :],
                        start=(c == 0),
                        stop=(c == 7),
                    )
                nc.scalar.activation(
                    out=h2Tb[:, f, :],
                    in_=h2ps[:, f, :],
                    func=AF.Relu,
                    bias=bcol(8 + f),
                )
            e2 = tmp.tile([P, 4, O], F32, tag="e2")
            l2ps = pwork.tile([P, 4, O], F32, tag="pwork", space="PSUM")
            for f in range(4):
                for c in range(4):
                    nc.tensor.matmul(
                        out=l2ps[:, f, :],
                        lhsT=wa_sb[:, c, f * P : (f + 1) * P],
                        rhs=h2Tb[:, c, :],
                        start=(c == 0),
                        stop=(c == 3),
                    )
                nc.scalar.activation(
                    out=e2[:, f, :],
                    in_=l2ps[:, f, :],
                    func=AF.Exp,
                    bias=bcol(12 + f),
                )

            # ---- classify head part 2: chunks 20..31 (run under the pair
            # softmax)
            for kc in kc_order[12:24]:
                nc.tensor.matmul(
                    out=pf,
                    lhsT=catFb[:, kc, :],
                    rhs=wl0_sb[:, kc, :],
                    start=False,
                    stop=False,
                )

            # pair softmax([l1[k], l2]) -> 1/na, 1/nb = s12 / (e1*a + e2*pool)
            raTb = act.tile([P, 4, O, 10], BF16)
            rbTb = act.tile([P, 4, O, 10], BF16)
            s12 = tmp.tile([P, 4, O, 10], F32, tag="s12")
            nc.vector.tensor_tensor(
                out=s12, in0=w, in1=e2.broadcast_to([P, 4, O, 10]), op=OP.add
            )
            for half, dstb in ((0, raTb), (1, rbTb)):
                q = tmp.tile([P, 4, O], F32, tag="q")
                nc.vector.tensor_tensor(
                    out=q, in0=e2, in1=cat2b[:, half * 4 : half * 4 + 4, :],
                    op=OP.mult,
                )
                t3 = tmp.tile([P, 4, O, 10], F32, tag="t3")
                nc.vector.tensor_tensor(
                    out=t3, in0=wts[half], in1=q.broadcast_to([P, 4, O, 10]),
                    op=OP.add,
                )
                it3 = tmp.tile([P, 4, O, 10], F32, tag="it3")
                nc.vector.reciprocal(out=it3, in_=t3)
                nc.vector.tensor_tensor(
                    out=dstb[:, :, :, :], in0=s12, in1=it3, op=OP.mult
                )

            # ---- union pool (weight-stationary, 40 rows)
            h3Tb = act.tile([P, 4, O, 10], BF16)
            h3ps = pwork.tile([P, 4, O * 10], F32, tag="pwork", space="PSUM")
            for f in range(4):
                for c in range(8):
                    src = raTb if c < 4 else rbTb
                    nc.tensor.matmul(
                        out=h3ps[:, f, :],
                        lhsT=wa0_sb[:, c, f * P : (f + 1) * P],
                        rhs=src[:, c % 4, :, :],
                        start=(c == 0),
                        stop=(c == 7),
                    )
                nc.scalar.activation(
                    out=h3Tb[:, f, :, :],
                    in_=h3ps.rearrange("p f (o k) -> p f o k", o=O)[:, f],
                    func=AF.Relu,
                    bias=bcol(8 + f),
                )
            w3 = tmp.tile([P, 4, O, 10], F32, tag="w3")
            l3ps = pwork.tile([P, 4, O * 10], F32, tag="pwork", space="PSUM")
            for f in range(4):
                for c in range(4):
                    nc.tensor.matmul(
                        out=l3ps[:, f, :],
                        lhsT=wa_sb[:, c, f * P : (f + 1) * P],
                        rhs=h3Tb[:, c, :, :],
                        start=(c == 0),
                        stop=(c == 3),
                    )
                nc.scalar.activation(
                    out=w3[:, f, :, :],
                    in_=l3ps.rearrange("p f (o k) -> p f o k", o=O)[:, f],
                    func=AF.Exp,
                    bias=bcol(12 + f),
                )

            # union softmax + weighted reduce + invert -> catFb chunks 0..7
            # ua = s3 / (sum_k w3 ra)
            s3 = tmp.tile([P, 4, O], F32, tag="s3")
            nc.vector.reduce_sum(s3, w3, axis=AX)
            for half, src in ((0, raTb), (1, rbTb)):
                tu = tmp.tile([P, 4, O, 10], F32, tag="tu")
                nc.vector.tensor_tensor(
                    out=tu, in0=w3, in1=src[:, :, :, :], op=OP.mult
                )
                su = tmp.tile([P, 4, O], F32, tag="su")
                nc.vector.reduce_sum(su, tu, axis=AX)
                isu = tmp.tile([P, 4, O], F32, tag="isu")
                nc.vector.reciprocal(out=isu, in_=su)
                nc.vector.tensor_tensor(
                    out=catFb[:, half * 4 : half * 4 + 4, :], in0=s3, in1=isu,
                    op=OP.mult,
                )

            # ---- classify head part 3: chunks 0..7 (ua/ub), close the
            # accumulation
            for i, kc in enumerate(kc_order[24:]):
                nc.tensor.matmul(
                    out=pf,
                    lhsT=catFb[:, kc, :],
                    rhs=wl0_sb[:, kc, :],
                    start=False,
                    stop=(i == 7),
                )
            # out = relu(hf) . Wl + bl: one max + one fused mul-reduce
            hrelu = rowsp.tile([O, 512], F32, tag="hrelu")
            nc.vector.tensor_scalar_max(out=hrelu, in0=pf[:, :], scalar1=0.0)
            hw = rowsp.tile([O, 512], F32, tag="hw")
            nc.vector.tensor_tensor(out=hw, in0=hrelu, in1=wlr_sb, op=OP.mult)
            osum = rowsp.tile([O, 1], F32, tag="osum")
            nc.vector.reduce_sum(osum, hw, axis=AX)
            out_sb = rowsp.tile([O, 1], F32, tag="out_sb")
            nc.vector.tensor_scalar_add(
                out=out_sb, in0=osum, scalar1=bias_sb[0:O, 20:21]
            )
            nc.sync.dma_start(out=out_d[:], in_=out_sb)

            if debug:
                for name, t, dt in (
                    ("xT", xT, BF16),
                    ("eTb", eTb, BF16),
                    ("w", w, F32),
                    ("cat2b", cat2b, BF16),
                    ("raTb", raTb, BF16),
                    ("rbTb", rbTb, BF16),
                    ("catFb", catFb, BF16),
                ):
                    d = nc.dram_tensor(
                        "dbg_" + name, list(t.shape), dt, kind="ExternalOutput"
                    )
                    nc.sync.dma_start(out=d[:], in_=t)

    _split_excess_waits(nc)
    return nc


_NC = None


def _get_nc():
    global _NC
    if _NC is None:
        _NC = _build_nc()
    return _NC


def _prep_inputs(hidden, idx, Wp, bp, Wa0, ba0, Wa, ba, Wl0, bl0, Wl, bl):
    hidden = np.asarray(hidden, dtype=np.float32)
    idx = np.asarray(idx).astype(np.int64)

    f32 = lambda a: np.ascontiguousarray(np.asarray(a, dtype=np.float32))
    bf = lambda a: np.ascontiguousarray(np.asarray(a, dtype=np.float32).astype(NPBF16))
    bp, ba0, ba, bl0, bl = f32(bp), f32(ba0), f32(ba), f32(bl0), f32(bl)
    Wl = f32(Wl)

    hid_b = np.ascontiguousarray(hidden.astype(NPHDT))  # [B, O, L, E]
    wp_t = bf(np.asarray(Wp, np.float32).reshape(8, P, 1024).transpose(1, 0, 2))
    wa0_t = bf(np.asarray(Wa0, np.float32).reshape(8, P, 512).transpose(1, 0, 2))
    wa_t = bf(np.asarray(Wa, np.float32).reshape(4, P, 512).transpose(1, 0, 2))
    wl0_t = bf(np.asarray(Wl0, np.float32).reshape(32, P, 512).transpose(1, 0, 2))

    biases = np.zeros((P, 21), dtype=np.float32)
    biases[:, 0:8] = (bp + 1.0).reshape(8, P).T
    biases[:, 8:12] = ba0.reshape(4, P).T
    biases[:, 12:16] = ba.reshape(4, P).T
    biases[:, 16:20] = bl0.reshape(4, P).T
    biases[:, 20] = bl[0]

    bl0rep = bf(np.broadcast_to(bl0 / 128.0, (P, 512)))
    wlrep = np.ascontiguousarray(np.broadcast_to(Wl[:, 0], (O, 512)).astype(np.float32))

    in_maps = []
    for b in range(B):
        m = np.zeros((L, NK), dtype=np.float32)
        cntinv = np.zeros((NK, 1), dtype=np.float32)
        ib = idx[b]
        starts = [1] + [int(ib[k]) for k in range(9)]
        ends = [int(ib[k]) for k in range(10)]
        segs = [(starts[k], ends[k]) for k in range(10)]
        segs.append((int(ib[9]), int(ib[10])))
        segs.append((int(ib[10]), int(ib[11])))
        segs.append((1, int(ib[9])))
        for k, (s, e) in enumerate(segs):
            m[s:e, k] = 1.0
            cntinv[k, 0] = 1.0 / (e - s)
        # l = p*T + t layout (matches the hidden SBUF tiling)
        maskt = np.ascontiguousarray(m.reshape(P, T, NK).astype(NPHDT))

        in_maps.append(
            dict(
                hidden=np.ascontiguousarray(hid_b[b]),
                maskt=maskt,
                cntinv=cntinv,
                wp=wp_t,
                wa0=wa0_t,
                wa=wa_t,
                wl0=wl0_t,
                biases=biases,
                bl0rep=bl0rep,
                wlrep=wlrep,
            )
        )
    return in_maps


def _run(in_maps, **kwargs):
    return run_bass_kernel_spmd(_get_nc(), in_maps, core_ids=list(range(B)), **kwargs)


def kernel(**inputs):
    in_maps = _prep_inputs(**inputs)
    res = _run(in_maps)
    return np.stack([r["out"].reshape(O, 1) for r in res.results])


def _install_ntff_hook():
    """The RL container's antenv lacks axon_hooks, so boot() skipped NTFF
    hook registration. Recreate the module and register the ctypes hook."""
    import sys
    import types

    name = "antenv.axon_hooks"
    if name not in sys.modules:
        try:
            __import__(name)
        except ImportError:
            mod = types.ModuleType(name)
            mod._hook = None
            mod.set_axon_ntff_profile_hook = lambda h: setattr(mod, "_hook", h)
            mod.get_axon_ntff_profile_hook = lambda: mod._hook
            sys.modules[name] = mod
            import antenv

            antenv.axon_hooks = mod
    import antenv.axon_hooks as ah

    if ah.get_axon_ntff_profile_hook() is None:
        from trn_agent_boot.trn_boot import _ntff_profile_via_ctypes

        ah.set_axon_ntff_profile_hook(
            _ntff_profile_via_ctypes("/opt/axon/libaxon_pjrt.so")
        )

    import concourse.bass_utils as bu

    bu.upload_artifacts = lambda tmpdir: tmpdir


def benchmark(trace_cores=None, **inputs):
    """Run with NTFF tracing; returns (output, BassKernelResults)."""
    _install_ntff_hook()
    in_maps = _prep_inputs(**inputs)
    res = _run(in_maps, trace=True, trace_cores=trace_cores)
    out = np.stack([r["out"].reshape(O, 1) for r in res.results])
    return out, res
